# revision 7
# baseline (speedup 1.0000x reference)
"""Trainium2 Bass kernel for nn_Loss_orthogonal: mean(x1 @ x2^T).

Algebraic identity: mean(x1 @ x2^T) = dot(colsum(x1), colsum(x2)) / N^2.
Each of the 8 cores reduces its 1/8 row-shard of x1 and x2 to per-column
partial sums; the host sums the 8 partials (in float64) and takes the tiny
dot product.

Per-core kernel (DMA-bound; ~8 MB of HBM reads at ~360 GB/s):
  - 8 row-tile loads [128, 1024] per matrix on the SP HWDGE ring
    (back-to-back, saturating the per-core HBM bandwidth),
  - a sequential fp32 accumulation chain on the (otherwise idle) vector
    engine as tiles arrive; the last tile's add is column-split so the
    first final matmul can start half a tile earlier,
  - partition-reduction via two fp32 ones-vector matmuls into one
    2-bank PSUM tile (fp32 PE matmuls are bit-faithful enough that the
    result matches the f32 reference to ~1e-7),
  - one ACT copy PSUM->SBUF and a per-matrix output DMA so matrix-1's
    store is fully hidden under matrix-2's loads.

All arithmetic is fp32 (no fp32r / bf16 shortcuts), keeping the result
numerically indistinguishable from the jax f32 reference.

Self-contained: hardcodes N=8192, D=1024, 8 cores; takes FULL inputs and
returns the FULL (scalar) output.
"""

import numpy as np

import concourse.mybir as mybir
import concourse.tile as tile
from concourse import bacc
from concourse.bass_utils import run_bass_kernel_spmd

N, D = 8192, 1024
N_CORES = 8
R = N // N_CORES        # 1024 rows per core
P = 128                 # SBUF partitions
N_RT = R // P           # 8 row-tiles per matrix per core
FH = 512                # free-dim half (one fp32 PSUM bank)

_NC_CACHE = None


def _build():
    global _NC_CACHE
    if _NC_CACHE is not None:
        return _NC_CACHE

    nc = bacc.Bacc(trn_type="TRN2", debug=False)
    x1 = nc.dram_tensor("x1", [R, D], mybir.dt.float32, kind="ExternalInput")
    x2 = nc.dram_tensor("x2", [R, D], mybir.dt.float32, kind="ExternalInput")
    out = nc.dram_tensor("out", [1, 2 * D], mybir.dt.float32,
                         kind="ExternalOutput")

    with tile.TileContext(nc) as tc:
        ones = nc.const_aps.tensor(1.0, [P, 1], mybir.dt.float32)
        with (
            tc.tile_pool(name="ld", bufs=2 * N_RT) as pool,
            tc.tile_pool(name="acc", bufs=2) as acc_pool,
            tc.tile_pool(name="ps", bufs=1, space="PSUM") as psum_pool,
            tc.tile_pool(name="ob", bufs=2) as opool,
        ):
            for m, x in enumerate((x1, x2)):
                xr = x.ap().rearrange("(n p) d -> p n d", p=P)
                tiles = []
                for i in range(N_RT):
                    t = pool.tile([P, 1, D], mybir.dt.float32, tag="ld",
                                  name=f"ld_{m}_{i}")
                    nc.sync.dma_start(out=t[:], in_=xr[:, i:i + 1, :])
                    tiles.append(t[:, 0, :])

                acc = acc_pool.tile([P, D], mybir.dt.float32, tag="acc",
                                    name=f"acc_{m}")
                nc.vector.tensor_add(acc[:], tiles[0], tiles[1])
                for t_ap in tiles[2:-1]:
                    nc.vector.tensor_add(acc[:], acc[:], t_ap)

                ps = psum_pool.tile([1, D], mybir.dt.float32,
                                    name=f"ps_{m}", tag=f"ps_{m}")
                last = tiles[-1]
                for h in range(D // FH):
                    sl = slice(h * FH, (h + 1) * FH)
                    nc.vector.tensor_add(acc[:, sl], acc[:, sl], last[:, sl])
                    nc.tensor.matmul(ps[0:1, sl], ones, acc[:, sl],
                                     start=True, stop=True)

                osb = opool.tile([1, D], mybir.dt.float32, tag="ob",
                                 name=f"osb_{m}")
                nc.scalar.copy(osb[:], ps[0:1, :])
                nc.sync.dma_start(out=out.ap()[0:1, m * D:(m + 1) * D],
                                  in_=osb[:])
    nc.compile()
    _NC_CACHE = nc
    return nc


def kernel(**inputs) -> np.ndarray:
    x1 = np.ascontiguousarray(np.asarray(inputs["x1"], dtype=np.float32))
    x2 = np.ascontiguousarray(np.asarray(inputs["x2"], dtype=np.float32))
    assert x1.shape == (N, D) and x2.shape == (N, D)

    nc = _build()
    in_maps = [
        {"x1": x1[c * R:(c + 1) * R], "x2": x2[c * R:(c + 1) * R]}
        for c in range(N_CORES)
    ]
    res = run_bass_kernel_spmd(nc, in_maps, core_ids=list(range(N_CORES)))

    parts = np.stack(
        [r["out"].reshape(2, D) for r in res.results]
    ).astype(np.float64)                       # [8, 2, D]
    sums = parts.sum(axis=0)                   # [2, D]
    ort = np.dot(sums[0], sums[1]) / (float(N) * float(N))
    return np.asarray(np.float32(ort))


# revision 8
# speedup vs baseline: 1.0053x; 1.0053x over previous
"""Trainium2 Bass kernel for nn_Loss_orthogonal: mean(x1 @ x2^T).

Algebraic identity: mean(x1 @ x2^T) = dot(colsum(x1), colsum(x2)) / N^2.
Each of the 8 cores reduces its 1/8 row-shard of x1 and x2 to per-column
partial sums; the host sums the 8 partials (in float64) and takes the tiny
dot product.

Per-core kernel (DMA-bound; ~8 MB of HBM reads at ~360 GB/s):
  - 8 row-tile loads [128, 1024] per matrix on the SP HWDGE ring
    (back-to-back, saturating the per-core HBM bandwidth),
  - a sequential fp32 accumulation chain on the (otherwise idle) vector
    engine as tiles arrive; the last tile's add is column-split so the
    first final matmul can start half a tile earlier,
  - partition-reduction via two fp32 ones-vector matmuls into one
    2-bank PSUM tile (fp32 PE matmuls are bit-faithful enough that the
    result matches the f32 reference to ~1e-7),
  - one ACT copy PSUM->SBUF and a per-matrix output DMA so matrix-1's
    store is fully hidden under matrix-2's loads.

All arithmetic is fp32 (no fp32r / bf16 shortcuts), keeping the result
numerically indistinguishable from the jax f32 reference.

Self-contained: hardcodes N=8192, D=1024, 8 cores; takes FULL inputs and
returns the FULL (scalar) output.
"""

import numpy as np

import concourse.mybir as mybir
import concourse.tile as tile
from concourse import bacc
from concourse.bass_utils import run_bass_kernel_spmd

N, D = 8192, 1024
N_CORES = 8
R = N // N_CORES        # 1024 rows per core
P = 128                 # SBUF partitions
N_RT = R // P           # 8 row-tiles per matrix per core
FH = 512                # free-dim half (one fp32 PSUM bank)

_NC_CACHE = None


def _build():
    global _NC_CACHE
    if _NC_CACHE is not None:
        return _NC_CACHE

    nc = bacc.Bacc(trn_type="TRN2", debug=False)
    x1 = nc.dram_tensor("x1", [R, D], mybir.dt.float32, kind="ExternalInput")
    x2 = nc.dram_tensor("x2", [R, D], mybir.dt.float32, kind="ExternalInput")
    out = nc.dram_tensor("out", [1, 2 * D], mybir.dt.float32,
                         kind="ExternalOutput")

    with tile.TileContext(nc) as tc:
        ones = nc.const_aps.tensor(1.0, [P, 1], mybir.dt.float32)
        with (
            tc.tile_pool(name="ld", bufs=2 * N_RT) as pool,
            tc.tile_pool(name="acc", bufs=2) as acc_pool,
            tc.tile_pool(name="ps", bufs=1, space="PSUM") as psum_pool,
            tc.tile_pool(name="ob", bufs=2) as opool,
        ):
            for m, x in enumerate((x1, x2)):
                xr = x.ap().rearrange("(n p) d -> p n d", p=P)
                tiles = []
                for i in range(N_RT - 1):
                    t = pool.tile([P, 1, D], mybir.dt.float32, tag="ld",
                                  name=f"ld_{m}_{i}")
                    nc.sync.dma_start(out=t[:], in_=xr[:, i:i + 1, :])
                    tiles.append(t[:, 0, :])
                # Last row-tile arrives as two column-half DMAs so the tail
                # add+matmul for the first half starts half a transfer early.
                tl = pool.tile([P, 1, D], mybir.dt.float32, tag="ld",
                               name=f"ld_{m}_last")
                for h in range(D // FH):
                    sl = slice(h * FH, (h + 1) * FH)
                    nc.sync.dma_start(out=tl[:, :, sl],
                                      in_=xr[:, N_RT - 1:N_RT, sl])

                acc = acc_pool.tile([P, D], mybir.dt.float32, tag="acc",
                                    name=f"acc_{m}")
                nc.vector.tensor_add(acc[:], tiles[0], tiles[1])
                for t_ap in tiles[2:]:
                    nc.vector.tensor_add(acc[:], acc[:], t_ap)

                ps = psum_pool.tile([1, D], mybir.dt.float32,
                                    name=f"ps_{m}", tag=f"ps_{m}")
                for h in range(D // FH):
                    sl = slice(h * FH, (h + 1) * FH)
                    nc.vector.tensor_add(acc[:, sl], acc[:, sl],
                                         tl[:, 0, sl])
                    nc.tensor.matmul(ps[0:1, sl], ones, acc[:, sl],
                                     start=True, stop=True)

                osb = opool.tile([1, D], mybir.dt.float32, tag="ob",
                                 name=f"osb_{m}")
                nc.scalar.copy(osb[:], ps[0:1, :])
                nc.sync.dma_start(out=out.ap()[0:1, m * D:(m + 1) * D],
                                  in_=osb[:])
    nc.compile()
    _NC_CACHE = nc
    return nc


def kernel(**inputs) -> np.ndarray:
    x1 = np.ascontiguousarray(np.asarray(inputs["x1"], dtype=np.float32))
    x2 = np.ascontiguousarray(np.asarray(inputs["x2"], dtype=np.float32))
    assert x1.shape == (N, D) and x2.shape == (N, D)

    nc = _build()
    in_maps = [
        {"x1": x1[c * R:(c + 1) * R], "x2": x2[c * R:(c + 1) * R]}
        for c in range(N_CORES)
    ]
    res = run_bass_kernel_spmd(nc, in_maps, core_ids=list(range(N_CORES)))

    parts = np.stack(
        [r["out"].reshape(2, D) for r in res.results]
    ).astype(np.float64)                       # [8, 2, D]
    sums = parts.sum(axis=0)                   # [2, D]
    ort = np.dot(sums[0], sums[1]) / (float(N) * float(N))
    return np.asarray(np.float32(ort))


# revision 9
# speedup vs baseline: 1.0674x; 1.0618x over previous
"""Trainium2 Bass kernel for nn_Loss_orthogonal: mean(x1 @ x2^T).

Algebraic identity: mean(x1 @ x2^T) = dot(colsum(x1), colsum(x2)) / N^2.
Each of the 8 cores reduces its 1/8 row-shard of x1 and x2 to per-column
partial sums; the host sums the 8 partials (in float64) and takes the tiny
dot product.

Per-core kernel (DMA-bound; ~8 MB of HBM reads at ~360 GB/s ≈ 23 us):
  - 8 row-tile loads [128, 1024] per matrix, back-to-back on the SP HWDGE
    ring; the last tile arrives as two column-half DMAs so tail work starts
    half a transfer early,
  - row-tile accumulation split across two otherwise-idle engines: the
    vector engine owns columns [0:512], GPSIMD owns [512:1024], so the two
    final adds run in parallel after the last byte lands,
  - partition-reduction per 128-column block via PE transpose
    (is_transpose matmul, 2 cyc/row fp32 — half the cost of a fp32
    ones-matmul) into PSUM, then one DVE reduce_sum per half straight into
    SBUF (no PSUM->SBUF ACT copy needed),
  - per-half output DMAs on the ACT HWDGE ring (keeps the input-DMA ring
    free of head-of-line blocking; matrix-1's stores hide under matrix-2's
    loads).

All arithmetic is fp32 (no fp32r / bf16 shortcuts); result matches the
jax f32 reference to ~1e-7.

Per-core output layout: out[m, c, j] = colsum of matrix m, column j*128+c
(c = PSUM partition after the block-j transpose).

Self-contained: hardcodes N=8192, D=1024, 8 cores; takes FULL inputs and
returns the FULL (scalar) output.
"""

import numpy as np

import concourse.mybir as mybir
import concourse.tile as tile
from concourse import bacc
from concourse.bass_utils import run_bass_kernel_spmd
from concourse.masks import make_identity

N, D = 8192, 1024
N_CORES = 8
R = N // N_CORES        # 1024 rows per core
P = 128                 # SBUF partitions
N_RT = R // P           # 8 row-tiles per matrix per core
FH = 512                # column half owned by each accumulation engine
N_BLK = D // P          # 8 transpose blocks
HB = N_BLK // 2         # blocks per half

_NC_CACHE = None


def _build():
    global _NC_CACHE
    if _NC_CACHE is not None:
        return _NC_CACHE

    nc = bacc.Bacc(trn_type="TRN2", debug=False)
    x1 = nc.dram_tensor("x1", [R, D], mybir.dt.float32, kind="ExternalInput")
    x2 = nc.dram_tensor("x2", [R, D], mybir.dt.float32, kind="ExternalInput")
    out = nc.dram_tensor("out", [2, P, N_BLK], mybir.dt.float32,
                         kind="ExternalOutput")

    with tile.TileContext(nc) as tc:
        with (
            tc.tile_pool(name="ld", bufs=2 * N_RT) as pool,
            tc.tile_pool(name="acc", bufs=2) as acc_pool,
            tc.tile_pool(name="ps", bufs=2, space="PSUM") as psum_pool,
            tc.tile_pool(name="ob", bufs=2) as opool,
        ):
            ident = acc_pool.tile([P, P], mybir.dt.float32, name="ident",
                                  tag="ident")
            make_identity(nc, ident[:])

            for m, x in enumerate((x1, x2)):
                xr = x.ap().rearrange("(n p) d -> p n d", p=P)
                tiles = []
                for i in range(N_RT - 1):
                    t = pool.tile([P, 1, D], mybir.dt.float32, tag="ld",
                                  name=f"ld_{m}_{i}")
                    nc.sync.dma_start(out=t[:], in_=xr[:, i:i + 1, :])
                    tiles.append(t[:, 0, :])
                # Last row-tile as two column-half DMAs: each engine's final
                # add starts as soon as its own half lands.
                tl = pool.tile([P, 1, D], mybir.dt.float32, tag="ld",
                               name=f"ld_{m}_last")
                for h in range(2):
                    sl = slice(h * FH, (h + 1) * FH)
                    nc.sync.dma_start(out=tl[:, :, sl],
                                      in_=xr[:, N_RT - 1:N_RT, sl])

                acc = acc_pool.tile([P, D], mybir.dt.float32, tag="acc",
                                    name=f"acc_{m}")
                eng_h = {0: nc.vector, 1: nc.gpsimd}
                for h in range(2):
                    sl = slice(h * FH, (h + 1) * FH)
                    e = eng_h[h]
                    e.tensor_add(acc[:, sl], tiles[0][:, sl], tiles[1][:, sl])
                    for t_ap in tiles[2:]:
                        e.tensor_add(acc[:, sl], acc[:, sl], t_ap[:, sl])
                    e.tensor_add(acc[:, sl], acc[:, sl], tl[:, 0, sl])

                ps = psum_pool.tile([P, N_BLK, P], mybir.dt.float32,
                                    name=f"pst_{m}", tag=f"pst_{m}")
                osb = opool.tile([P, N_BLK], mybir.dt.float32, tag="ob",
                                 name=f"osb_{m}")
                for h in range(2):
                    for j in range(h * HB, (h + 1) * HB):
                        nc.tensor.transpose(
                            ps[:, j, :], acc[:, j * P:(j + 1) * P], ident[:]
                        )
                    nc.vector.reduce_sum(
                        out=osb[:, h * HB:(h + 1) * HB],
                        in_=ps[:, h * HB:(h + 1) * HB, :],
                        axis=mybir.AxisListType.X,
                    )
                    nc.scalar.dma_start(
                        out=out.ap()[m, :, h * HB:(h + 1) * HB],
                        in_=osb[:, h * HB:(h + 1) * HB],
                    )
    nc.compile()
    _NC_CACHE = nc
    return nc


def kernel(**inputs) -> np.ndarray:
    x1 = np.ascontiguousarray(np.asarray(inputs["x1"], dtype=np.float32))
    x2 = np.ascontiguousarray(np.asarray(inputs["x2"], dtype=np.float32))
    assert x1.shape == (N, D) and x2.shape == (N, D)

    nc = _build()
    in_maps = [
        {"x1": x1[c * R:(c + 1) * R], "x2": x2[c * R:(c + 1) * R]}
        for c in range(N_CORES)
    ]
    res = run_bass_kernel_spmd(nc, in_maps, core_ids=list(range(N_CORES)))

    # out[m, c, j] = colsum_m[j*128 + c]  ->  [n_cores, 2, D]
    parts = np.stack([r["out"] for r in res.results]).astype(np.float64)
    colsums = parts.transpose(0, 1, 3, 2).reshape(N_CORES, 2, D).sum(axis=0)
    ort = np.dot(colsums[0], colsums[1]) / (float(N) * float(N))
    return np.asarray(np.float32(ort))


# revision 10
# speedup vs baseline: 1.1249x; 1.0538x over previous
"""Trainium2 Bass kernel for nn_Loss_orthogonal: mean(x1 @ x2^T).

Algebraic identity: mean(x1 @ x2^T) = dot(colsum(x1), colsum(x2)) / N^2.
Each of the 8 cores reduces its 1/8 row-shard of x1 and x2 to per-column
partial sums; the host sums the 8 partials (in float64) and takes the tiny
dot product.

Per-core kernel (DMA-bound; ~8 MB of HBM reads at ~360 GB/s ≈ 23 us):
  - 8 row-tile loads [128, 1024] per matrix, back-to-back on the SP HWDGE
    ring; the last tile arrives as two column-half DMAs so tail work starts
    half a transfer early,
  - row-tile accumulation split across two otherwise-idle engines: the
    vector engine owns columns [0:512], GPSIMD owns [512:1024]. GPSIMD's
    fp32 add (~1.46 us incl. handoff per [128,512] tile) exactly matches
    the DMA cadence, so its chain accumulates lag; therefore x1 donates
    its first three h1-adds to the DVE (Pool starts x1 late and finishes
    early) and x2's FINAL h1-add runs on the DVE, so the saturated GPSIMD
    chain never gates the tail,
  - x1 (hidden under x2's input stream): partition-reduce on device via
    PE transpose per 128-column block (is_transpose matmul, 2 cyc/row
    fp32) into PSUM + one DVE reduce_sum per half straight into SBUF,
    stored as [128, 8] on the ACT HWDGE ring,
  - x2 (the tail matrix): NO on-device partition-reduce — the [128, 1024]
    accumulator ships raw as two 256 KB stores (the DMA engines are idle
    once the input stream ends, and two stores cost less than
    transpose+reduce+store); the host finishes the 128-way partition sum
    in float64, which is both faster and more accurate.

All device arithmetic is fp32 (no fp32r / bf16 shortcuts); result matches
the jax f32 reference to ~1e-7.

Per-core outputs:
  out  [128, 8]   : x1 colsums, out[c, j] = colsum1[j*128 + c]
  out2 [128, 1024]: x2 accumulator, colsum2[d] = sum_p out2[p, d]

Self-contained: hardcodes N=8192, D=1024, 8 cores; takes FULL inputs and
returns the FULL (scalar) output.
"""

import numpy as np

import concourse.mybir as mybir
import concourse.tile as tile
from concourse import bacc
from concourse.bass_utils import run_bass_kernel_spmd
from concourse.masks import make_identity

N, D = 8192, 1024
N_CORES = 8
R = N // N_CORES        # 1024 rows per core
P = 128                 # SBUF partitions
N_RT = R // P           # 8 row-tiles per matrix per core
FH = 512                # column half owned by each accumulation engine
N_BLK = D // P          # 8 transpose blocks
HB = N_BLK // 2         # blocks per half

_NC_CACHE = None


def _build():
    global _NC_CACHE
    if _NC_CACHE is not None:
        return _NC_CACHE

    nc = bacc.Bacc(trn_type="TRN2", debug=False)
    x1 = nc.dram_tensor("x1", [R, D], mybir.dt.float32, kind="ExternalInput")
    x2 = nc.dram_tensor("x2", [R, D], mybir.dt.float32, kind="ExternalInput")
    out = nc.dram_tensor("out", [P, N_BLK], mybir.dt.float32,
                         kind="ExternalOutput")
    out2 = nc.dram_tensor("out2", [P, D], mybir.dt.float32,
                          kind="ExternalOutput")

    sl0, sl1 = slice(0, FH), slice(FH, D)
    with tile.TileContext(nc) as tc:
        with (
            tc.tile_pool(name="ld", bufs=2 * N_RT) as pool,
            tc.tile_pool(name="acc", bufs=2) as acc_pool,
            tc.tile_pool(name="ps", bufs=2, space="PSUM") as psum_pool,
            tc.tile_pool(name="ob", bufs=2) as opool,
        ):
            ident = acc_pool.tile([P, P], mybir.dt.float32, name="ident",
                                  tag="ident")
            make_identity(nc, ident[:])

            for m, x in enumerate((x1, x2)):
                xr = x.ap().rearrange("(n p) d -> p n d", p=P)
                tiles = []
                for i in range(N_RT - 1):
                    t = pool.tile([P, 1, D], mybir.dt.float32, tag="ld",
                                  name=f"ld_{m}_{i}")
                    nc.sync.dma_start(out=t[:], in_=xr[:, i:i + 1, :])
                    tiles.append(t[:, 0, :])
                # Last row-tile as two column-half DMAs.
                tl = pool.tile([P, 1, D], mybir.dt.float32, tag="ld",
                               name=f"ld_{m}_last")
                for h in range(2):
                    sl = slice(h * FH, (h + 1) * FH)
                    nc.sync.dma_start(out=tl[:, :, sl],
                                      in_=xr[:, N_RT - 1:N_RT, sl])
                tiles.append(tl[:, 0, :])

                acc = acc_pool.tile([P, D], mybir.dt.float32, tag="acc",
                                    name=f"acc_{m}")
                # h0 chain fully on DVE.
                nc.vector.tensor_add(acc[:, sl0], tiles[0][:, sl0],
                                     tiles[1][:, sl0])
                for t_ap in tiles[2:]:
                    nc.vector.tensor_add(acc[:, sl0], acc[:, sl0],
                                         t_ap[:, sl0])
                # h1 chain on GPSIMD, with load-balancing exceptions.
                if m == 0:
                    nc.vector.tensor_add(acc[:, sl1], tiles[0][:, sl1],
                                         tiles[1][:, sl1])
                    nc.vector.tensor_add(acc[:, sl1], acc[:, sl1],
                                         tiles[2][:, sl1])
                    nc.vector.tensor_add(acc[:, sl1], acc[:, sl1],
                                         tiles[3][:, sl1])
                    for t_ap in tiles[4:]:
                        nc.gpsimd.tensor_add(acc[:, sl1], acc[:, sl1],
                                             t_ap[:, sl1])
                else:
                    nc.gpsimd.tensor_add(acc[:, sl1], tiles[0][:, sl1],
                                         tiles[1][:, sl1])
                    for t_ap in tiles[2:-1]:
                        nc.gpsimd.tensor_add(acc[:, sl1], acc[:, sl1],
                                             t_ap[:, sl1])
                    nc.vector.tensor_add(acc[:, sl1], acc[:, sl1],
                                         tiles[-1][:, sl1])

                if m == 0:
                    ps = psum_pool.tile([P, N_BLK, P], mybir.dt.float32,
                                        name="pst_0", tag="pst_0")
                    osb = opool.tile([P, N_BLK], mybir.dt.float32, tag="ob",
                                     name="osb_0")
                    for h in range(2):
                        for j in range(h * HB, (h + 1) * HB):
                            nc.tensor.transpose(
                                ps[:, j, :], acc[:, j * P:(j + 1) * P],
                                ident[:]
                            )
                        nc.vector.reduce_sum(
                            out=osb[:, h * HB:(h + 1) * HB],
                            in_=ps[:, h * HB:(h + 1) * HB, :],
                            axis=mybir.AxisListType.X,
                        )
                        nc.scalar.dma_start(
                            out=out.ap()[:, h * HB:(h + 1) * HB],
                            in_=osb[:, h * HB:(h + 1) * HB],
                        )
                else:
                    for h in range(2):
                        sl = slice(h * FH, (h + 1) * FH)
                        nc.scalar.dma_start(out=out2.ap()[:, sl],
                                            in_=acc[:, sl])
    nc.compile()
    _NC_CACHE = nc
    return nc


def kernel(**inputs) -> np.ndarray:
    x1 = np.ascontiguousarray(np.asarray(inputs["x1"], dtype=np.float32))
    x2 = np.ascontiguousarray(np.asarray(inputs["x2"], dtype=np.float32))
    assert x1.shape == (N, D) and x2.shape == (N, D)

    nc = _build()
    in_maps = [
        {"x1": x1[c * R:(c + 1) * R], "x2": x2[c * R:(c + 1) * R]}
        for c in range(N_CORES)
    ]
    res = run_bass_kernel_spmd(nc, in_maps, core_ids=list(range(N_CORES)))

    cs1 = np.zeros(D, dtype=np.float64)
    cs2 = np.zeros(D, dtype=np.float64)
    for r in res.results:
        cs1 += r["out"].astype(np.float64).T.reshape(D)
        cs2 += r["out2"].astype(np.float64).sum(axis=0)
    ort = np.dot(cs1, cs2) / (float(N) * float(N))
    return np.asarray(np.float32(ort))


# revision 12
# speedup vs baseline: 1.1479x; 1.0204x over previous
"""Trainium2 Bass kernel for nn_Loss_orthogonal: mean(x1 @ x2^T).

Algebraic identity: mean(x1 @ x2^T) = dot(colsum(x1), colsum(x2)) / N^2.
Each of the 8 cores reduces its 1/8 row-shard of x1 and x2 to per-column
partial sums; the host sums the 8 partials (in float64) and takes the tiny
dot product.

Per-core kernel (DMA-bound; ~8 MB of HBM reads at ~360 GB/s ≈ 23 us):
  - 8 row-tile loads [128, 1024] per matrix, back-to-back on the SP HWDGE
    ring; the last tile arrives as two column-half DMAs so tail work starts
    half a transfer early,
  - row-tile accumulation split across two otherwise-idle engines: the
    vector engine owns columns [0:512], GPSIMD owns [512:1024]. GPSIMD's
    fp32 add (~1.46 us incl. handoff per [128,512] tile) exactly matches
    the DMA cadence, so its chain accumulates lag; therefore x1 donates
    its first three h1-adds to the DVE (Pool starts x1 late and finishes
    early) and x2's FINAL h1-add runs on the DVE, so the saturated GPSIMD
    chain never gates the tail,
  - x1 (hidden under x2's input stream): partition-reduce on device via
    PE transpose per 128-column block (is_transpose matmul, 2 cyc/row
    fp32) into PSUM + one DVE reduce_sum per half straight into SBUF,
    stored as [128, 8] on the ACT HWDGE ring,
  - x2 (the tail matrix): NO on-device partition-reduce — the [128, 1024]
    accumulator ships raw as two 256 KB stores (the DMA engines are idle
    once the input stream ends, and two stores cost less than
    transpose+reduce+store); the host finishes the 128-way partition sum
    in float64, which is both faster and more accurate.

Additionally, the Bass-init preamble (four const-tile memsets + an
all-engine barrier) is stripped from the entry block before compile: this
kernel never reads the const tiles (the BIR verifier itself flags them as
reader-less), the barrier's gather/release semaphores are self-contained,
and every body instruction carries its own Tile-generated waits — removing
it lets the first input DMA issue ~0.6 us earlier.

All device arithmetic is fp32 (no fp32r / bf16 shortcuts); result matches
the jax f32 reference to ~1e-7.

Per-core outputs:
  out  [128, 8]   : x1 colsums, out[c, j] = colsum1[j*128 + c]
  out2 [128, 1024]: x2 accumulator, colsum2[d] = sum_p out2[p, d]

Self-contained: hardcodes N=8192, D=1024, 8 cores; takes FULL inputs and
returns the FULL (scalar) output.
"""

import numpy as np

import concourse.mybir as mybir
import concourse.tile as tile
from concourse import bacc
from concourse.bass_utils import run_bass_kernel_spmd
from concourse.masks import make_identity

N, D = 8192, 1024
N_CORES = 8
R = N // N_CORES        # 1024 rows per core
P = 128                 # SBUF partitions
N_RT = R // P           # 8 row-tiles per matrix per core
FH = 512                # column half owned by each accumulation engine
N_BLK = D // P          # 8 transpose blocks
HB = N_BLK // 2         # blocks per half

_NC_CACHE = None


def _build():
    global _NC_CACHE
    if _NC_CACHE is not None:
        return _NC_CACHE

    nc = bacc.Bacc(trn_type="TRN2", debug=False)
    x1 = nc.dram_tensor("x1", [R, D], mybir.dt.float32, kind="ExternalInput")
    x2 = nc.dram_tensor("x2", [R, D], mybir.dt.float32, kind="ExternalInput")
    out = nc.dram_tensor("out", [P, N_BLK], mybir.dt.float32,
                         kind="ExternalOutput")
    out2 = nc.dram_tensor("out2", [P, D], mybir.dt.float32,
                          kind="ExternalOutput")

    sl0, sl1 = slice(0, FH), slice(FH, D)
    with tile.TileContext(nc) as tc:
        with (
            tc.tile_pool(name="ld", bufs=2 * N_RT) as pool,
            tc.tile_pool(name="acc", bufs=2) as acc_pool,
            tc.tile_pool(name="ps", bufs=2, space="PSUM") as psum_pool,
            tc.tile_pool(name="ob", bufs=2) as opool,
        ):
            ident = acc_pool.tile([P, P], mybir.dt.float32, name="ident",
                                  tag="ident")
            make_identity(nc, ident[:])

            for m, x in enumerate((x1, x2)):
                xr = x.ap().rearrange("(n p) d -> p n d", p=P)
                tiles = []
                for i in range(N_RT - 1):
                    t = pool.tile([P, 1, D], mybir.dt.float32, tag="ld",
                                  name=f"ld_{m}_{i}")
                    nc.sync.dma_start(out=t[:], in_=xr[:, i:i + 1, :])
                    tiles.append(t[:, 0, :])
                # Last row-tile as two column-half DMAs.
                tl = pool.tile([P, 1, D], mybir.dt.float32, tag="ld",
                               name=f"ld_{m}_last")
                for h in range(2):
                    sl = slice(h * FH, (h + 1) * FH)
                    nc.sync.dma_start(out=tl[:, :, sl],
                                      in_=xr[:, N_RT - 1:N_RT, sl])
                tiles.append(tl[:, 0, :])

                acc = acc_pool.tile([P, D], mybir.dt.float32, tag="acc",
                                    name=f"acc_{m}")
                # h0 chain fully on DVE.
                nc.vector.tensor_add(acc[:, sl0], tiles[0][:, sl0],
                                     tiles[1][:, sl0])
                for t_ap in tiles[2:]:
                    nc.vector.tensor_add(acc[:, sl0], acc[:, sl0],
                                         t_ap[:, sl0])
                # h1 chain on GPSIMD, with load-balancing exceptions.
                if m == 0:
                    nc.vector.tensor_add(acc[:, sl1], tiles[0][:, sl1],
                                         tiles[1][:, sl1])
                    nc.vector.tensor_add(acc[:, sl1], acc[:, sl1],
                                         tiles[2][:, sl1])
                    nc.vector.tensor_add(acc[:, sl1], acc[:, sl1],
                                         tiles[3][:, sl1])
                    for t_ap in tiles[4:]:
                        nc.gpsimd.tensor_add(acc[:, sl1], acc[:, sl1],
                                             t_ap[:, sl1])
                else:
                    nc.gpsimd.tensor_add(acc[:, sl1], tiles[0][:, sl1],
                                         tiles[1][:, sl1])
                    for t_ap in tiles[2:-1]:
                        nc.gpsimd.tensor_add(acc[:, sl1], acc[:, sl1],
                                             t_ap[:, sl1])
                    nc.vector.tensor_add(acc[:, sl1], acc[:, sl1],
                                         tiles[-1][:, sl1])

                if m == 0:
                    ps = psum_pool.tile([P, N_BLK, P], mybir.dt.float32,
                                        name="pst_0", tag="pst_0")
                    osb = opool.tile([P, N_BLK], mybir.dt.float32, tag="ob",
                                     name="osb_0")
                    for h in range(2):
                        for j in range(h * HB, (h + 1) * HB):
                            nc.tensor.transpose(
                                ps[:, j, :], acc[:, j * P:(j + 1) * P],
                                ident[:]
                            )
                        nc.vector.reduce_sum(
                            out=osb[:, h * HB:(h + 1) * HB],
                            in_=ps[:, h * HB:(h + 1) * HB, :],
                            axis=mybir.AxisListType.X,
                        )
                        nc.scalar.dma_start(
                            out=out.ap()[:, h * HB:(h + 1) * HB],
                            in_=osb[:, h * HB:(h + 1) * HB],
                        )
                else:
                    for h in range(2):
                        sl = slice(h * FH, (h + 1) * FH)
                        nc.scalar.dma_start(out=out2.ap()[:, sl],
                                            in_=acc[:, sl])

    # Strip the Bass preamble (const-tile memsets + init all-engine
    # barrier) from the entry block — dead code for this kernel; see
    # module docstring.
    fn = nc.m.functions[0]
    blocks = fn.blocks if isinstance(fn.blocks, list) else list(fn.blocks.values())
    b0 = blocks[0]
    keep = []
    for ins in b0.instructions:
        tn = type(ins).__name__
        if tn == "InstMemset":
            continue
        if tn in ("InstDrain", "InstEventSemaphore") and (
            ins.name.startswith("barrier_")
            or (ins.name.startswith("I-") and int(ins.name[2:]) < 60)
        ):
            continue
        keep.append(ins)
    b0.instructions[:] = keep

    nc.compile()
    _NC_CACHE = nc
    return nc


def kernel(**inputs) -> np.ndarray:
    x1 = np.ascontiguousarray(np.asarray(inputs["x1"], dtype=np.float32))
    x2 = np.ascontiguousarray(np.asarray(inputs["x2"], dtype=np.float32))
    assert x1.shape == (N, D) and x2.shape == (N, D)

    nc = _build()
    in_maps = [
        {"x1": x1[c * R:(c + 1) * R], "x2": x2[c * R:(c + 1) * R]}
        for c in range(N_CORES)
    ]
    res = run_bass_kernel_spmd(nc, in_maps, core_ids=list(range(N_CORES)))

    cs1 = np.zeros(D, dtype=np.float64)
    cs2 = np.zeros(D, dtype=np.float64)
    for r in res.results:
        cs1 += r["out"].astype(np.float64).T.reshape(D)
        cs2 += r["out2"].astype(np.float64).sum(axis=0)
    ort = np.dot(cs1, cs2) / (float(N) * float(N))
    return np.asarray(np.float32(ort))
